# revision 1
# baseline (speedup 1.0000x reference)
"""Multi-head self-attention (B=2, S=4096, D=512, H=8, Dk=64) on 8 TRN2 cores.

Sharding: data-parallel over batch x head-parallel. Core c handles batch
c//4 and head pair (2*(c%4), 2*(c%4)+1). Each core computes Q/K/V
projections for its 128 model dims, full attention for its two heads, and
a partial output projection against its 128 rows of Wo. The host sums the
four partial outputs per batch and adds bo.

x arrives host-transposed as xT [512, S] bf16, so the on-device load is
plain chunked DMAs (no S2M xbar transposes); projections are interleaved
with the chunk stream so the PE follows the DMA instead of waiting for
all of x.

On-core layout (bf16 operands, fp32 psum accumulation):
  xT   [d, s]   bf16, host-transposed        (rhs for Q/K, lhsT for V)
  QT/KT [128, S] bf16, head0 in partitions 0:64, head1 in 64:128
  V    [s, 128] bf16, head0 in cols 0:64, head1 in 64:128 (lhsT for ctx)
  scoresT[k, q] fp32 psum from row-packed bf16 matmul pairs (K=64/head)
  attnT = exp(scoresT/8 + mask_bias) bf16, one ACT op per [128, 1024] block
  ctxT [d, q] fp32 psum, col-packed over k blocks; denominators from
  ones-vector matmuls into psum rows 0/32; normalization via fp32 PE
  broadcast of the reciprocals.
"""

import numpy as np
import ml_dtypes
from contextlib import ExitStack

import concourse.bass as bass
import concourse.tile as tile
from concourse import bacc, mybir
from concourse.bass_utils import run_bass_kernel_spmd
from concourse.tile_rust import add_dep_helper

F32 = mybir.dt.float32
F32R = mybir.dt.float32r
F16 = mybir.dt.float16
BF16 = mybir.dt.bfloat16
EXP = mybir.ActivationFunctionType.Exp

D_MODEL = 512
N_HEADS = 8
D_K = 64
N_CORES = 8
DL = 128          # local model dims per core (2 heads)
Q_BLK = 512       # query block (free dim of scores matmuls)
SCALE = 1.0 / np.sqrt(D_K).item()


def build_kernel(ctx, tc, S, use_mask, use_bq, use_bk, use_bv, d):
    nc = tc.nc
    SB = S // 128    # s blocks of 128
    QB = S // Q_BLK  # query blocks of 512
    KB = S // 128    # key blocks of 128

    sp = ctx.enter_context(tc.tile_pool(name="sp", bufs=1))
    psum = ctx.enter_context(tc.tile_pool(name="psum", bufs=1, space="PSUM"))
    # psum budget (8 banks): scores 2x[128,1024]=4, ctx 2x[128,512]=2,
    # den 2x[<=1 bank]=2. All other matmul outputs share the ctx/den tags.

    # ---- constants ----
    ones_f = sp.tile([128, 1], F32, tag="ones_f")
    nc.vector.memset(ones_f, 1.0)
    ones_col = sp.tile([128, 1], BF16, tag="ones_col")  # lhsT of denominator mms
    nc.vector.tensor_copy(ones_col, ones_f)
    ones_rep = sp.tile([33, 128], F16, tag="ones_rep")  # lhsT of broadcast mms
    nc.vector.memset(ones_rep, 1.0)

    # ---- phase 1: DMA in. x arrives HOST-TRANSPOSED as xT [512, S] bf16,
    # so the load is plain DMAs (no S2M xbar transposes, no serialization
    # hazard), streamed in 1024-token chunks so projections can start as
    # soon as the first chunk lands. Weights go first (small; needed by
    # every projection). ----
    wq_sb = sp.tile([128, 4, 128], BF16, tag="wq")
    nc.sync.dma_start(wq_sb, d["wq"].ap().rearrange("(t p) d -> p t d", p=128))
    wk_sb = sp.tile([128, 4, 128], BF16, tag="wk")
    nc.sync.dma_start(wk_sb, d["wk"].ap().rearrange("(t p) d -> p t d", p=128))
    wv_sb = sp.tile([128, 4, 128], BF16, tag="wv")
    nc.sync.dma_start(wv_sb, d["wv"].ap().rearrange("(t p) d -> p t d", p=128))
    wo_sb = sp.tile([128, 512], BF16, tag="wo")
    nc.sync.dma_start(wo_sb, d["wo"].ap())
    if use_bq:
        bq_sb = sp.tile([128, 1], F32, tag="bq")
        nc.sync.dma_start(bq_sb, d["bq"].ap()[:, None])
    if use_bk:
        bk_sb = sp.tile([128, 1], F32, tag="bk")
        nc.sync.dma_start(bk_sb, d["bk"].ap()[:, None])
    if use_bv:
        bv_sb = sp.tile([1, 128], F32, tag="bv")
        nc.sync.dma_start(bv_sb, d["bv"].ap()[None, :])
        ones_row = sp.tile([1, 128], F32, tag="ones_row")
        nc.vector.memset(ones_row, 1.0)
    if use_mask:
        mb_sb = sp.tile([128, KB], F32, tag="mb")
        nc.sync.dma_start(mb_sb, d["mb"].ap())

    CHUNK = 1024
    NCH = S // CHUNK
    xt = sp.tile([128, 4, S], BF16, tag="xt")
    xsrc = d["xt"].ap().rearrange("(t p) s -> p t s", p=128)
    for c in range(NCH):
        cs = slice(c * CHUNK, (c + 1) * CHUNK)
        nc.sync.dma_start(xt[:, :, cs], xsrc[:, :, cs])

    # ---- PE warm-up: the HAM clock gate needs ~3.4us of sustained matmul
    # activity to lift the PE from 1.2 to 2.4 GHz; run throwaway matmuls
    # while x streams in so the projections start at full clock. ----
    scratch = sp.tile([128, 512], BF16, tag="scratch")
    nc.vector.memset(scratch, 0.0)
    for _ in range(24):
        pw = psum.tile([33, 512], F32, tag="den", bufs=2, name="pw")
        nc.tensor.matmul(pw[0:1, :], scratch[:, 0:1], scratch)

    # ---- phase 2: projections, interleaved by x chunk so the PE follows
    # the DMA stream instead of waiting for all of x ----
    qt = sp.tile([128, S], BF16, tag="qt")
    kt = sp.tile([128, S], BF16, tag="kt")
    v_all = sp.tile([128, SB, 128], BF16, tag="v")
    for c in range(NCH):
        for dst, w_sb, b_sb in (
            (kt, wk_sb, bk_sb if use_bk else None),
            (qt, wq_sb, bq_sb if use_bq else None),
        ):
            for sc in range(c * CHUNK // 512, (c + 1) * CHUNK // 512):
                pp = psum.tile([128, 512], F32, tag="ctx", bufs=2)
                for t in range(4):
                    nc.tensor.matmul(
                        pp, w_sb[:, t, :], xt[:, t, sc * 512:(sc + 1) * 512],
                        start=(t == 0), stop=(t == 3))
                out = dst[:, sc * 512:(sc + 1) * 512]
                if b_sb is not None:
                    nc.vector.tensor_scalar_add(out, pp, b_sb[:, 0:1])
                else:
                    nc.vector.tensor_copy(out, pp)
        for sb in range(c * CHUNK // 128, (c + 1) * CHUNK // 128):
            pv = psum.tile([128, 128], F32, tag="den", bufs=2)
            for t in range(4):
                nc.tensor.matmul(
                    pv, xt[:, t, sb * 128:(sb + 1) * 128], wv_sb[:, t, :],
                    start=(t == 0), stop=(t == 3 and not use_bv))
            if use_bv:
                nc.tensor.matmul(pv, ones_row[0:1, :], bv_sb[0:1, :],
                                 start=False, stop=True)
            nc.vector.tensor_copy(v_all[:, sb, :], pv)

    # ---- phase 3: attention ----
    ctxn = sp.tile([128, S], BF16, tag="ctxn")
    pending_tail = {}
    for qb in range(QB):
        qs = slice(qb * Q_BLK, (qb + 1) * Q_BLK)
        pc = psum.tile([128, 512], F32, tag="ctx", bufs=2)
        pd = psum.tile([33, 512], F32, tag="den", bufs=2)

        def scores_block(kb):
            # one query-block column of scores for both heads + its exp
            ks = slice(kb * 128, (kb + 1) * 128)
            ps = psum.tile([128, 1024], F32, tag="scores", bufs=2, name="ps")
            nc.tensor.matmul(ps[:, 0:512], kt[0:64, ks], qt[0:64, qs])
            nc.tensor.matmul(ps[:, 512:1024], kt[64:128, ks], qt[64:128, qs])
            attn = sp.tile([128, 1024], BF16, tag="attn", bufs=4, name="attn")
            nc.scalar.activation(
                attn, ps, EXP, scale=SCALE,
                bias=mb_sb[:, kb:kb + 1] if use_mask else 0.0)
            return attn

        # Software-pipelined: scores/exp for kb+1 are emitted before the
        # ctx/den matmuls of kb, so the PE streams scores(kb+1) while the
        # ACT engine computes exp(kb) — the serial exp->ctx->scores->exp
        # chain would otherwise set the loop period.
        attn = scores_block(0)
        for kb in range(KB):
            stage = pending_tail.pop(kb, None)
            if stage is not None:
                stage()
            attn_next = scores_block(kb + 1) if kb + 1 < KB else None
            first, last = kb == 0, kb == KB - 1
            nc.tensor.matmul(pc[0:64, :], v_all[:, kb, 0:64],
                             attn[:, 0:512], start=first, stop=last,
                             skip_group_check=True)
            nc.tensor.matmul(pc[64:128, :], v_all[:, kb, 64:128],
                             attn[:, 512:1024], start=first, stop=last,
                             skip_group_check=True)
            nc.tensor.matmul(pd[0:1, :], ones_col[:, 0:1],
                             attn[:, 0:512], start=first, stop=last,
                             skip_group_check=True)
            nc.tensor.matmul(pd[32:33, :], ones_col[:, 0:1],
                             attn[:, 512:1024], start=first, stop=last,
                             skip_group_check=True)
            attn = attn_next

        # Denominator extraction + reciprocal start immediately (DVE is
        # idle during the matmul loop). One reciprocal op spans both rows
        # (cost is free-dim based; rows 1..31 are memset, never consumed).
        den_sb = sp.tile([33, 512], F32, tag="den_sb", bufs=2)
        nc.vector.memset(den_sb, 1.0)
        nc.vector.tensor_copy(den_sb[0:1, :], pd[0:1, :])
        nc.vector.tensor_copy(den_sb[32:33, :], pd[32:33, :])
        rcp_f = sp.tile([33, 512], F32, tag="rcp_f", bufs=2)
        nc.vector.reciprocal(rcp_f, den_sb)
        rcp = sp.tile([33, 512], F16, tag="rcp", bufs=2)
        nc.vector.tensor_copy(rcp, rcp_f)

        # The PE/DVE parts of the tail (broadcast matmuls, normalize,
        # output projection) are STAGGERED across qb+1's iterations: each
        # piece is <=1us of PE work emitted at a different iteration top,
        # so the scores pipeline never drains and the ACT engine (the
        # throughput floor) keeps streaming through the qb boundary.
        reps = [None, None]

        def bcast(h, rcp=rcp, reps=reps):
            r = 32 * h
            pr = psum.tile([128, 512], F32, tag="den", bufs=2, name=f"pr{h}")
            nc.tensor.matmul(pr, ones_rep[r:r + 1, :], rcp[r:r + 1, :])
            rep = sp.tile([128, 512], F32, tag="rep", bufs=2, name=f"rep{h}")
            nc.vector.tensor_copy(rep, pr)
            reps[h] = rep

        def muls(qs=qs, pc=pc, reps=reps):
            nc.vector.tensor_mul(ctxn[0:64, qs], pc[0:64, :], reps[0][0:64, :])
            nc.vector.tensor_mul(ctxn[64:128, qs], pc[64:128, :],
                                 reps[1][64:128, :])

        def oproj(i, qb=qb):
            sb = qb * (Q_BLK // 128) + i
            po = psum.tile([128, 512], F32, tag="ctx", bufs=2, name="po")
            nc.tensor.matmul(po, ctxn[:, sb * 128:(sb + 1) * 128], wo_sb)
            ob = sp.tile([128, 512], F32, tag="ob", bufs=3, name="ob")
            nc.vector.tensor_copy(ob, po)
            nc.sync.dma_start(d["out"].ap()[sb * 128:(sb + 1) * 128, :], ob)

        pending_tail = {
            8: lambda: bcast(0),
            11: lambda: bcast(1),
            14: muls,
            17: lambda: oproj(0),
            20: lambda: oproj(1),
            23: lambda: oproj(2),
            26: lambda: oproj(3),
        }

    # final tail after the last query block
    for kb in sorted(pending_tail):
        pending_tail[kb]()


def build_program(S=4096, use_mask=False, use_bq=False, use_bk=False,
                  use_bv=False, enable_asserts=False):
    nc = bacc.Bacc("TRN2", target_bir_lowering=False, debug=False,
                   enable_asserts=enable_asserts, num_devices=N_CORES,
                   name="mha")
    d = {
        "xt": nc.dram_tensor("xt", [D_MODEL, S], BF16, kind="ExternalInput"),
        "wq": nc.dram_tensor("wq", [D_MODEL, DL], BF16, kind="ExternalInput"),
        "wk": nc.dram_tensor("wk", [D_MODEL, DL], BF16, kind="ExternalInput"),
        "wv": nc.dram_tensor("wv", [D_MODEL, DL], BF16, kind="ExternalInput"),
        "wo": nc.dram_tensor("wo", [DL, D_MODEL], BF16, kind="ExternalInput"),
        "out": nc.dram_tensor("out", [S, D_MODEL], F32, kind="ExternalOutput"),
    }
    if use_bq:
        d["bq"] = nc.dram_tensor("bq", [DL], F32, kind="ExternalInput")
    if use_bk:
        d["bk"] = nc.dram_tensor("bk", [DL], F32, kind="ExternalInput")
    if use_bv:
        d["bv"] = nc.dram_tensor("bv", [DL], F32, kind="ExternalInput")
    if use_mask:
        d["mb"] = nc.dram_tensor("mb", [128, S // 128], F32,
                                 kind="ExternalInput")
    with tile.TileContext(nc) as tc:
        with ExitStack() as ctx:
            build_kernel(ctx, tc, S, use_mask, use_bq, use_bk, use_bv, d)
    nc.compile()
    return nc


_cache = {}


def _program(key):
    if key not in _cache:
        _cache[key] = build_program(
            S=4096, use_mask=key[0], use_bq=key[1], use_bk=key[2],
            use_bv=key[3])
    return _cache[key]


def kernel(x, mask, Wq, bq, Wk, bk, Wv, bv, Wo, bo, _results_hook=None):
    x = np.asarray(x, np.float32)
    mask = np.asarray(mask)
    B, S, _ = x.shape
    use_mask = bool((mask == 0).any())
    use_bq = bool(np.asarray(bq).any())
    use_bk = bool(np.asarray(bk).any())
    use_bv = bool(np.asarray(bv).any())
    nc = _program((use_mask, use_bq, use_bk, use_bv))

    in_maps = []
    for c in range(N_CORES):
        b, j = divmod(c, N_CORES // B)
        ds = slice(j * DL, (j + 1) * DL)
        m = {
            "xt": np.ascontiguousarray(x[b].T).astype(ml_dtypes.bfloat16),
            "wq": np.ascontiguousarray(Wq[:, ds]).astype(ml_dtypes.bfloat16),
            "wk": np.ascontiguousarray(Wk[:, ds]).astype(ml_dtypes.bfloat16),
            "wv": np.ascontiguousarray(Wv[:, ds]).astype(ml_dtypes.bfloat16),
            "wo": np.ascontiguousarray(Wo[ds, :]).astype(ml_dtypes.bfloat16),
        }
        if use_bq:
            m["bq"] = np.ascontiguousarray(bq[ds], dtype=np.float32)
        if use_bk:
            m["bk"] = np.ascontiguousarray(bk[ds], dtype=np.float32)
        if use_bv:
            m["bv"] = np.ascontiguousarray(bv[ds], dtype=np.float32)
        if use_mask:
            mb = np.where(np.asarray(mask[b]) == 0, -1e9, 0.0).astype(np.float32)
            m["mb"] = np.ascontiguousarray(mb.reshape(S // 128, 128).T)
        in_maps.append(m)

    res = run_bass_kernel_spmd(nc, in_maps, core_ids=list(range(N_CORES)))
    if _results_hook is not None:
        _results_hook(res)
    out = np.zeros((B, S, D_MODEL), np.float32)
    for c in range(N_CORES):
        b = c // (N_CORES // B)
        out[b] += res.results[c]["out"]
    out += np.asarray(bo, np.float32)
    return out



# revision 2
# speedup vs baseline: 1.0021x; 1.0021x over previous
"""Multi-head self-attention (B=2, S=4096, D=512, H=8, Dk=64) on 8 TRN2 cores.

Sharding: data-parallel over batch x head-parallel. Core c handles batch
c//4 and head pair (2*(c%4), 2*(c%4)+1). Each core computes Q/K/V
projections for its 128 model dims, full attention for its two heads, and
a partial output projection against its 128 rows of Wo. The host sums the
four partial outputs per batch and adds bo.

x arrives host-transposed as xT [512, S] bf16, streamed in 512-token
chunks; chunk-0 projections run up front and the remaining chunks'
K/Q/V projections are staggered into query-block 0's key loop so
attention (and the exp stream, the critical resource) starts as soon as
chunk 0 lands instead of after all projections.

The softmax exp is split across TWO engines: the Scalar (ACT) engine
computes exact exp for most key blocks, and the Vector (DVE) engine
computes a Schraudolph-style exp2 approximation (one tensor_scalar op:
round(s*A + B) -> int16, bit-cast as bf16) for DVE_QB blocks per query
block. ACT throughput is (N+352)/1.2 ns per [128, N] block and exp
exists only on ACT, so offloading ~40% of blocks to the otherwise-idle
DVE removes the single-engine exp floor (~294 us). The approximation
carries +-3% per-weight error; softmax renormalization cancels the mean
and the verified end-to-end rel err is ~1.3e-2 (gate 2e-2).

On-core layout (bf16 operands, fp32 psum accumulation):
  xT   [d, s]   bf16, host-transposed        (rhs for Q/K, lhsT for V)
  QT/KT [128, S] bf16, head0 in partitions 0:64, head1 in 64:128
  V    [s, 128] bf16, head0 in cols 0:64, head1 in 64:128 (lhsT for ctx)
  scoresT[k, q] fp32 psum from row-paired bf16 matmuls (K=64/head)
  attnT = exp(scoresT/8) bf16, per [128, 1024] block on ACT or DVE
  ctxT [d, q] fp32 psum, col-paired over k blocks; denominators from
  ones-vector matmuls into psum rows 0/32; reciprocal_approx_fast reads
  them straight from psum; normalization via fp32 PE broadcast.
"""

import numpy as np
import ml_dtypes
from contextlib import ExitStack

import concourse.bass as bass
import concourse.tile as tile
from concourse import bacc, mybir
from concourse.bass_utils import run_bass_kernel_spmd
from concourse.tile_rust import add_dep_helper

F32 = mybir.dt.float32
F16 = mybir.dt.float16
BF16 = mybir.dt.bfloat16
I16 = mybir.dt.int16
EXP = mybir.ActivationFunctionType.Exp

D_MODEL = 512
N_HEADS = 8
D_K = 64
N_CORES = 8
DL = 128          # local model dims per core (2 heads)
Q_BLK = 512       # query block (free dim of scores matmuls)
SCALE = 1.0 / np.sqrt(D_K).item()

# Schraudolph exp2 on DVE: exp(s/8) ~ bf16-bitcast(int16(round(s*A + B)))
LOG2E = 1.4426950408889634
SCH_A = 128.0 * LOG2E * SCALE
SCH_B = 128.0 * (127.0 - 0.0434)

# key blocks per query block handled by the DVE exp path
DVE_KBS_QB0 = (5, 9, 13, 17, 21, 25)
DVE_KBS = tuple(range(1, 28, 2))  # 14 of 32


def build_kernel(ctx, tc, S, use_mask, use_bq, use_bk, use_bv, d):
    nc = tc.nc
    SB = S // 128    # s blocks of 128
    QB = S // Q_BLK  # query blocks of 512
    KB = S // 128    # key blocks of 128
    CHUNK = 512
    NCH = S // CHUNK

    sp = ctx.enter_context(tc.tile_pool(name="sp", bufs=1))
    psum = ctx.enter_context(tc.tile_pool(name="psum", bufs=1, space="PSUM"))
    # psum budget (8 banks): scores 2x[128,1024]=4, ctx 2x[128,512]=2,
    # den 2x[<=1 bank]=2. All other matmul outputs share the ctx/den tags.

    # ---- constants ----
    ones_f = sp.tile([128, 1], F32, tag="ones_f")
    nc.vector.memset(ones_f, 1.0)
    ones_col = sp.tile([128, 1], BF16, tag="ones_col")  # lhsT of denominator mms
    nc.vector.tensor_copy(ones_col, ones_f)
    ones_rep = sp.tile([33, 128], F16, tag="ones_rep")  # lhsT of broadcast mms
    nc.vector.memset(ones_rep, 1.0)

    # ---- DMA in: weights first (small; needed by every projection), then
    # x (host-transposed bf16) in 512-token chunks so chunk-0 projections
    # and the first exp start as early as possible. ----
    wq_sb = sp.tile([128, 4, 128], BF16, tag="wq")
    nc.sync.dma_start(wq_sb, d["wq"].ap().rearrange("(t p) d -> p t d", p=128))
    wk_sb = sp.tile([128, 4, 128], BF16, tag="wk")
    nc.sync.dma_start(wk_sb, d["wk"].ap().rearrange("(t p) d -> p t d", p=128))
    wv_sb = sp.tile([128, 4, 128], BF16, tag="wv")
    nc.sync.dma_start(wv_sb, d["wv"].ap().rearrange("(t p) d -> p t d", p=128))
    wo_sb = sp.tile([128, 512], BF16, tag="wo")
    nc.sync.dma_start(wo_sb, d["wo"].ap())
    if use_bq:
        bq_sb = sp.tile([128, 1], F32, tag="bq")
        nc.sync.dma_start(bq_sb, d["bq"].ap()[:, None])
    if use_bk:
        bk_sb = sp.tile([128, 1], F32, tag="bk")
        nc.sync.dma_start(bk_sb, d["bk"].ap()[:, None])
    if use_bv:
        bv_sb = sp.tile([1, 128], F32, tag="bv")
        nc.sync.dma_start(bv_sb, d["bv"].ap()[None, :])
        ones_row = sp.tile([1, 128], F32, tag="ones_row")
        nc.vector.memset(ones_row, 1.0)
    if use_mask:
        mb_sb = sp.tile([128, KB], F32, tag="mb")
        nc.sync.dma_start(mb_sb, d["mb"].ap())

    xt = sp.tile([128, 4, S], BF16, tag="xt")
    xsrc = d["xt"].ap().rearrange("(t p) s -> p t s", p=128)
    for c in range(NCH):
        cs = slice(c * CHUNK, (c + 1) * CHUNK)
        nc.sync.dma_start(xt[:, :, cs], xsrc[:, :, cs])

    # ---- PE warm-up: the HAM clock gate needs ~3.4us of sustained matmul
    # activity to lift the PE from 1.2 to 2.4 GHz; run throwaway matmuls
    # while x streams in so the projections start at full clock. ----
    scratch = sp.tile([128, 512], BF16, tag="scratch")
    nc.vector.memset(scratch, 0.0)
    for _ in range(20):
        pw = psum.tile([33, 512], F32, tag="den", bufs=2, name="pw")
        nc.tensor.matmul(pw[0:1, :], scratch[:, 0:1], scratch)

    # ---- projections ----
    qt = sp.tile([128, S], BF16, tag="qt")
    kt = sp.tile([128, S], BF16, tag="kt")
    v_all = sp.tile([128, SB, 128], BF16, tag="v")

    def proj_qk(dst, w_sb, b_sb, c):
        # one 512-token sub-chunk of the Q or K projection
        pp = psum.tile([128, 512], F32, tag="ctx", bufs=2, name="pp")
        for t in range(4):
            nc.tensor.matmul(pp, w_sb[:, t, :], xt[:, t, c * 512:(c + 1) * 512],
                             start=(t == 0), stop=(t == 3))
        out = dst[:, c * 512:(c + 1) * 512]
        if b_sb is not None:
            nc.vector.tensor_scalar_add(out, pp, b_sb[:, 0:1])
        else:
            nc.vector.tensor_copy(out, pp)

    def proj_v(sb):
        pv = psum.tile([128, 128], F32, tag="den", bufs=2, name="pv")
        for t in range(4):
            nc.tensor.matmul(pv, xt[:, t, sb * 128:(sb + 1) * 128], wv_sb[:, t, :],
                             start=(t == 0), stop=(t == 3 and not use_bv))
        if use_bv:
            nc.tensor.matmul(pv, ones_row[0:1, :], bv_sb[0:1, :],
                             start=False, stop=True)
        nc.vector.tensor_copy(v_all[:, sb, :], pv)

    bkk = bk_sb if use_bk else None
    bqq = bq_sb if use_bq else None

    # chunk 0 up front: everything query-block 0 needs to start
    proj_qk(kt, wk_sb, bkk, 0)
    proj_qk(qt, wq_sb, bqq, 0)
    for sb in range(4):
        proj_v(sb)

    # chunks 1..NCH-1 staggered into qb0's key loop. Chunk c's K/V blocks
    # are first consumed at kb=4c; emit them 4 iterations ahead.
    def make_chunk_stage(c, part):
        if part == 0:
            return lambda: proj_qk(kt, wk_sb, bkk, c)
        if part == 1:
            return lambda: (proj_v(4 * c), proj_v(4 * c + 1))
        return lambda: (proj_v(4 * c + 2), proj_v(4 * c + 3),
                        proj_qk(qt, wq_sb, bqq, c))

    pending_tail = {}
    for c in range(1, NCH):
        base = 4 * (c - 1)
        for part in range(3):
            pending_tail.setdefault(base + part, []).append(
                make_chunk_stage(c, part))

    # ---- attention ----
    ctxn = sp.tile([128, S], BF16, tag="ctxn")
    for qb in range(QB):
        qs = slice(qb * Q_BLK, (qb + 1) * Q_BLK)
        pc = psum.tile([128, 512], F32, tag="ctx", bufs=2, name="pc")
        pd = psum.tile([33, 512], F32, tag="den", bufs=2, name="pd")
        dve_set = () if use_mask else (DVE_KBS_QB0 if qb == 0 else DVE_KBS)

        def scores_block(kb, qs=qs, dve_set=dve_set):
            # one query-block column of scores for both heads + its exp
            ks = slice(kb * 128, (kb + 1) * 128)
            ps = psum.tile([128, 1024], F32, tag="scores", bufs=2, name="ps")
            nc.tensor.matmul(ps[:, 0:512], kt[0:64, ks], qt[0:64, qs])
            nc.tensor.matmul(ps[:, 512:1024], kt[64:128, ks], qt[64:128, qs])
            attn = sp.tile([128, 1024], BF16, tag="attn", bufs=4, name="attn")
            if kb in dve_set:
                nc.vector.tensor_scalar(
                    attn.bitcast(I16), ps, SCH_A, SCH_B,
                    mybir.AluOpType.mult, mybir.AluOpType.add)
            else:
                nc.scalar.activation(
                    attn, ps, EXP, scale=SCALE,
                    bias=mb_sb[:, kb:kb + 1] if use_mask else 0.0)
            return attn

        # Software-pipelined: scores/exp for kb+1 are emitted before the
        # ctx/den matmuls of kb, so the PE streams scores(kb+1) while the
        # exp engines work on block kb.
        attn = scores_block(0)
        for kb in range(KB):
            stages = pending_tail.pop(kb, None)
            if stages is not None:
                for stage in stages:
                    stage()
            attn_next = scores_block(kb + 1) if kb + 1 < KB else None
            first, last = kb == 0, kb == KB - 1
            nc.tensor.matmul(pc[0:64, :], v_all[:, kb, 0:64],
                             attn[:, 0:512], start=first, stop=last,
                             skip_group_check=True)
            nc.tensor.matmul(pc[64:128, :], v_all[:, kb, 64:128],
                             attn[:, 512:1024], start=first, stop=last,
                             skip_group_check=True)
            nc.tensor.matmul(pd[0:1, :], ones_col[:, 0:1],
                             attn[:, 0:512], start=first, stop=last,
                             skip_group_check=True)
            nc.tensor.matmul(pd[32:33, :], ones_col[:, 0:1],
                             attn[:, 512:1024], start=first, stop=last,
                             skip_group_check=True)
            attn = attn_next

        # Denominator reciprocals straight from psum (rows 1..31 are psum
        # garbage, computed but never consumed; cost is free-dim based).
        rcp_f = sp.tile([33, 512], F32, tag="rcp_f", bufs=2)
        nc.vector.reciprocal_approx_fast(rcp_f, pd)
        rcp = sp.tile([33, 512], F16, tag="rcp", bufs=2)
        nc.vector.tensor_copy(rcp, rcp_f)

        # The PE/DVE parts of the tail (broadcast matmuls, normalize,
        # output projection) are STAGGERED across qb+1's iterations so the
        # scores pipeline never drains; for the last qb they run eagerly.
        reps = [None, None]

        def bcast(h, rcp=rcp, reps=reps):
            r = 32 * h
            pr = psum.tile([128, 512], F32, tag="den", bufs=2, name=f"pr{h}")
            nc.tensor.matmul(pr, ones_rep[r:r + 1, :], rcp[r:r + 1, :])
            rep = sp.tile([128, 512], F32, tag="rep", bufs=2, name=f"rep{h}")
            nc.vector.tensor_copy(rep, pr)
            reps[h] = rep

        def muls(qs=qs, pc=pc, reps=reps):
            nc.vector.tensor_mul(ctxn[0:64, qs], pc[0:64, :], reps[0][0:64, :])
            nc.vector.tensor_mul(ctxn[64:128, qs], pc[64:128, :],
                                 reps[1][64:128, :])

        def oproj(i, qb=qb):
            sb = qb * (Q_BLK // 128) + i
            po = psum.tile([128, 512], F32, tag="ctx", bufs=2, name="po")
            nc.tensor.matmul(po, ctxn[:, sb * 128:(sb + 1) * 128], wo_sb)
            ob = sp.tile([128, 512], F32, tag="ob", bufs=3, name="ob")
            nc.vector.tensor_copy(ob, po)
            nc.sync.dma_start(d["out"].ap()[sb * 128:(sb + 1) * 128, :], ob)

        tail = [lambda: bcast(0), lambda: bcast(1), muls,
                lambda: oproj(0), lambda: oproj(1),
                lambda: oproj(2), lambda: oproj(3)]
        if qb == QB - 1:
            for t in tail:
                t()
        else:
            for slot, t in zip((8, 11, 14, 17, 20, 23, 26), tail):
                pending_tail.setdefault(slot, []).append(t)

    # anything left (shouldn't be, but keep it correct)
    for kb in sorted(pending_tail):
        for stage in pending_tail[kb]:
            stage()


def build_program(S=4096, use_mask=False, use_bq=False, use_bk=False,
                  use_bv=False, enable_asserts=False):
    nc = bacc.Bacc("TRN2", target_bir_lowering=False, debug=False,
                   enable_asserts=enable_asserts, num_devices=N_CORES,
                   name="mha")
    d = {
        "xt": nc.dram_tensor("xt", [D_MODEL, S], BF16, kind="ExternalInput"),
        "wq": nc.dram_tensor("wq", [D_MODEL, DL], BF16, kind="ExternalInput"),
        "wk": nc.dram_tensor("wk", [D_MODEL, DL], BF16, kind="ExternalInput"),
        "wv": nc.dram_tensor("wv", [D_MODEL, DL], BF16, kind="ExternalInput"),
        "wo": nc.dram_tensor("wo", [DL, D_MODEL], BF16, kind="ExternalInput"),
        "out": nc.dram_tensor("out", [S, D_MODEL], F32, kind="ExternalOutput"),
    }
    if use_bq:
        d["bq"] = nc.dram_tensor("bq", [DL], F32, kind="ExternalInput")
    if use_bk:
        d["bk"] = nc.dram_tensor("bk", [DL], F32, kind="ExternalInput")
    if use_bv:
        d["bv"] = nc.dram_tensor("bv", [DL], F32, kind="ExternalInput")
    if use_mask:
        d["mb"] = nc.dram_tensor("mb", [128, S // 128], F32,
                                 kind="ExternalInput")
    with tile.TileContext(nc) as tc:
        with ExitStack() as ctx:
            build_kernel(ctx, tc, S, use_mask, use_bq, use_bk, use_bv, d)
    nc.compile()
    return nc


_cache = {}


def _program(key):
    if key not in _cache:
        _cache[key] = build_program(
            S=4096, use_mask=key[0], use_bq=key[1], use_bk=key[2],
            use_bv=key[3])
    return _cache[key]


def kernel(x, mask, Wq, bq, Wk, bk, Wv, bv, Wo, bo, _results_hook=None):
    x = np.asarray(x, np.float32)
    mask = np.asarray(mask)
    B, S, _ = x.shape
    use_mask = bool((mask == 0).any())
    use_bq = bool(np.asarray(bq).any())
    use_bk = bool(np.asarray(bk).any())
    use_bv = bool(np.asarray(bv).any())
    nc = _program((use_mask, use_bq, use_bk, use_bv))

    in_maps = []
    for c in range(N_CORES):
        b, j = divmod(c, N_CORES // B)
        ds = slice(j * DL, (j + 1) * DL)
        m = {
            "xt": np.ascontiguousarray(x[b].T).astype(ml_dtypes.bfloat16),
            "wq": np.ascontiguousarray(Wq[:, ds]).astype(ml_dtypes.bfloat16),
            "wk": np.ascontiguousarray(Wk[:, ds]).astype(ml_dtypes.bfloat16),
            "wv": np.ascontiguousarray(Wv[:, ds]).astype(ml_dtypes.bfloat16),
            "wo": np.ascontiguousarray(Wo[ds, :]).astype(ml_dtypes.bfloat16),
        }
        if use_bq:
            m["bq"] = np.ascontiguousarray(bq[ds], dtype=np.float32)
        if use_bk:
            m["bk"] = np.ascontiguousarray(bk[ds], dtype=np.float32)
        if use_bv:
            m["bv"] = np.ascontiguousarray(bv[ds], dtype=np.float32)
        if use_mask:
            mb = np.where(np.asarray(mask[b]) == 0, -1e9, 0.0).astype(np.float32)
            m["mb"] = np.ascontiguousarray(mb.reshape(S // 128, 128).T)
        in_maps.append(m)

    res = run_bass_kernel_spmd(nc, in_maps, core_ids=list(range(N_CORES)))
    if _results_hook is not None:
        _results_hook(res)
    out = np.zeros((B, S, D_MODEL), np.float32)
    for c in range(N_CORES):
        b = c // (N_CORES // B)
        out[b] += res.results[c]["out"]
    out += np.asarray(bo, np.float32)
    return out


# revision 8
# speedup vs baseline: 1.1258x; 1.1234x over previous
"""Multi-head self-attention (B=2, S=4096, D=512, H=8, Dk=64) on 8 TRN2 cores.

Sharding: data-parallel over batch x head-parallel. Core c handles batch
c//4 and head pair (2*(c%4), 2*(c%4)+1). Each core computes Q/K/V
projections for its 128 model dims, full attention for its two heads, and
a partial output projection against its 128 rows of Wo. The host sums the
four partial outputs per batch and adds bo.

x arrives host-transposed as xT [512, S] bf16, streamed in 512-token
chunks; chunk-0 projections run up front and the remaining chunks'
K/Q/V projections are staggered into query-block 0's key loop so
attention (and the exp stream, the critical resource) starts as soon as
chunk 0 lands instead of after all projections.

The softmax exp is split across TWO engines: the Scalar (ACT) engine
computes exact exp for most key blocks, and the Vector (DVE) engine
computes a Schraudolph-style exp2 approximation (one tensor_scalar op:
round(s*A + B) -> int16, bit-cast as bf16) for DVE_QB blocks per query
block. ACT throughput is (N+352)/1.2 ns per [128, N] block and exp
exists only on ACT, so offloading ~40% of blocks to the otherwise-idle
DVE removes the single-engine exp floor (~294 us). The approximation
carries +-3% per-weight error; softmax renormalization cancels the mean
and the verified end-to-end rel err is ~1.3e-2 (gate 2e-2).

On-core layout (bf16 operands, fp32 psum accumulation):
  xT   [d, s]   bf16, host-transposed        (rhs for Q/K, lhsT for V)
  QT/KT [128, S] bf16, head0 in partitions 0:64, head1 in 64:128
  V    [s, 128] bf16, head0 in cols 0:64, head1 in 64:128 (lhsT for ctx)
  scoresT[k, q] fp32 psum from row-paired bf16 matmuls (K=64/head)
  attnT = exp(scoresT/8) bf16, per [128, 1024] block on ACT or DVE
  ctxT [d, q] fp32 psum, col-paired over k blocks; denominators from
  ones-vector matmuls into psum rows 0/32; reciprocal_approx_fast reads
  them straight from psum; normalization via fp32 PE broadcast.
"""

import numpy as np
import ml_dtypes
from contextlib import ExitStack

import concourse.bass as bass
import concourse.tile as tile
from concourse import bacc, mybir
from concourse.bass_utils import run_bass_kernel_spmd
from concourse.tile_rust import add_dep_helper

F32 = mybir.dt.float32
F16 = mybir.dt.float16
BF16 = mybir.dt.bfloat16
I16 = mybir.dt.int16
EXP = mybir.ActivationFunctionType.Exp

D_MODEL = 512
N_HEADS = 8
D_K = 64
N_CORES = 8
DL = 128          # local model dims per core (2 heads)
Q_BLK = 512       # query block (free dim of scores matmuls)
SCALE = 1.0 / np.sqrt(D_K).item()

# Schraudolph exp2 on DVE: exp(s/8) ~ bf16-bitcast(int16(round(s*A + B)))
LOG2E = 1.4426950408889634
SCH_A = 128.0 * LOG2E * SCALE
SCH_B = 128.0 * (127.0 - 0.0434)

# key blocks per query block handled by the DVE exp path (odd kbs only:
# even blocks stay on ACT so the even psum-ring slot always frees in time)
DVE_KBS = tuple(range(3, 27, 2))  # 12 of 32


def build_kernel(ctx, tc, S, use_mask, use_bq, use_bk, use_bv, d):
    nc = tc.nc
    SB = S // 128    # s blocks of 128
    QB = S // Q_BLK  # query blocks of 512
    KB = S // 128    # key blocks of 128
    CHUNK = 512
    NCH = S // CHUNK

    sp = ctx.enter_context(tc.tile_pool(name="sp", bufs=1))
    psum = ctx.enter_context(tc.tile_pool(name="psum", bufs=1, space="PSUM"))
    # psum budget (8 banks): scores 2x[128,1024]=4, ctx 2x[128,512]=2,
    # den 2x[<=1 bank]=2. All other matmul outputs share the ctx/den tags.

    # ---- constants ----
    ones_f = sp.tile([128, 1], F32, tag="ones_f")
    nc.vector.memset(ones_f, 1.0)
    ones_col = sp.tile([128, 1], BF16, tag="ones_col")  # lhsT of denominator mms
    nc.vector.tensor_copy(ones_col, ones_f)
    ones_rep = sp.tile([33, 128], F16, tag="ones_rep")  # lhsT of broadcast mms
    nc.vector.memset(ones_rep, 1.0)
    # selector for the denominator-halves sum: dsum = sel.T @ pd_sb with
    # col 0 picking rows {0,64} (head 0) and col 1 rows {32,96} (head 1)
    sel = sp.tile([97, 2], F16, tag="sel")
    nc.vector.memset(sel, 0.0)
    nc.vector.memset(sel[0:1, 0:1], 1.0)
    nc.vector.memset(sel[64:65, 0:1], 1.0)
    nc.vector.memset(sel[32:33, 1:2], 1.0)
    nc.vector.memset(sel[96:97, 1:2], 1.0)

    # ---- DMA in: weights first (small; needed by every projection), then
    # x (host-transposed bf16) in 512-token chunks so chunk-0 projections
    # and the first exp start as early as possible. ----
    wq_sb = sp.tile([128, 4, 128], BF16, tag="wq")
    nc.sync.dma_start(wq_sb, d["wq"].ap().rearrange("(t p) d -> p t d", p=128))
    wk_sb = sp.tile([128, 4, 128], BF16, tag="wk")
    nc.sync.dma_start(wk_sb, d["wk"].ap().rearrange("(t p) d -> p t d", p=128))
    wv_sb = sp.tile([128, 4, 128], BF16, tag="wv")
    nc.sync.dma_start(wv_sb, d["wv"].ap().rearrange("(t p) d -> p t d", p=128))
    wo_sb = sp.tile([128, 512], BF16, tag="wo")
    nc.sync.dma_start(wo_sb, d["wo"].ap())
    if use_bq:
        bq_sb = sp.tile([128, 1], F32, tag="bq")
        nc.sync.dma_start(bq_sb, d["bq"].ap()[:, None])
    if use_bk:
        bk_sb = sp.tile([128, 1], F32, tag="bk")
        nc.sync.dma_start(bk_sb, d["bk"].ap()[:, None])
    if use_bv:
        bv_sb = sp.tile([1, 128], F32, tag="bv")
        nc.sync.dma_start(bv_sb, d["bv"].ap()[None, :])
        ones_row = sp.tile([1, 128], F32, tag="ones_row")
        nc.vector.memset(ones_row, 1.0)
    if use_mask:
        mb_sb = sp.tile([128, KB], F32, tag="mb")
        nc.sync.dma_start(mb_sb, d["mb"].ap())

    xt = sp.tile([128, 4, S], BF16, tag="xt")
    xsrc = d["xt"].ap().rearrange("(t p) s -> p t s", p=128)
    for c in range(NCH):
        cs = slice(c * CHUNK, (c + 1) * CHUNK)
        nc.sync.dma_start(xt[:, :, cs], xsrc[:, :, cs])

    # ---- PE warm-up: the HAM clock gate needs ~3.4us of sustained matmul
    # activity to lift the PE from 1.2 to 2.4 GHz; run throwaway matmuls
    # while x streams in so the projections start at full clock. ----
    scratch = sp.tile([128, 512], BF16, tag="scratch")
    nc.vector.memset(scratch, 0.0)
    for _ in range(20):
        pw = psum.tile([33, 512], F32, tag="den", bufs=2, name="pw")
        nc.tensor.matmul(pw[0:1, :], scratch[:, 0:1], scratch)

    # ---- projections ----
    qt = sp.tile([128, S], BF16, tag="qt")
    kt = sp.tile([128, S], BF16, tag="kt")
    v_all = sp.tile([128, SB, 128], BF16, tag="v")

    def proj_qk(dst, w_sb, b_sb, c):
        # one 512-token sub-chunk of the Q or K projection
        pp = psum.tile([128, 512], F32, tag="ctx", bufs=2, name="pp")
        for t in range(4):
            nc.tensor.matmul(pp, w_sb[:, t, :], xt[:, t, c * 512:(c + 1) * 512],
                             start=(t == 0), stop=(t == 3))
        out = dst[:, c * 512:(c + 1) * 512]
        if b_sb is not None:
            nc.vector.tensor_scalar_add(out, pp, b_sb[:, 0:1])
        else:
            nc.vector.tensor_copy(out, pp)

    def proj_v(sb):
        pv = psum.tile([128, 128], F32, tag="den", bufs=2, name="pv")
        for t in range(4):
            nc.tensor.matmul(pv, xt[:, t, sb * 128:(sb + 1) * 128], wv_sb[:, t, :],
                             start=(t == 0), stop=(t == 3 and not use_bv))
        if use_bv:
            nc.tensor.matmul(pv, ones_row[0:1, :], bv_sb[0:1, :],
                             start=False, stop=True)
        nc.vector.tensor_copy(v_all[:, sb, :], pv)

    bkk = bk_sb if use_bk else None
    bqq = bq_sb if use_bq else None

    # chunk 0 up front: everything query-block 0 needs to start
    proj_qk(kt, wk_sb, bkk, 0)
    proj_qk(qt, wq_sb, bqq, 0)
    for sb in range(4):
        proj_v(sb)

    # chunks 1..NCH-1 staggered into qb0's units. Chunk c's K/V blocks are
    # first consumed at global block 4c (scores emitted at unit 2c-2), so
    # emit them in the two units before that, DMA-gated by Tile deps.
    pending = {}

    def make_chunk_stage(c, part):
        if part == 0:
            return lambda: (proj_qk(kt, wk_sb, bkk, c),
                            proj_v(4 * c), proj_v(4 * c + 1))
        return lambda: (proj_v(4 * c + 2), proj_v(4 * c + 3),
                        proj_qk(qt, wq_sb, bqq, c))

    for c in range(1, NCH):
        u = 2 * (c - 1)
        pending.setdefault(u, []).append(make_chunk_stage(c, 0))
        pending.setdefault(u + 1, []).append(make_chunk_stage(c, 1))

    # ---- attention: one flat loop over 2-block units ----
    # Per unit (blocks g0=2u, g1=2u+1): ctx pair for g0, scores pair for
    # g0+4, ctx pair for g1, scores pair for g1+4, then ONE fused 4-way
    # denominator slot (M=1 matmuls on col groups 0/32/64/96 of pd).
    # The 4-block scores lookahead keeps exp latency off the ctx critical
    # path; even blocks always use ACT so the even psum-ring slot is
    # ready when scores(g+4) needs it.
    NBLK = QB * KB
    ctxn = sp.tile([128, S], BF16, tag="ctxn")

    def scores_block(g):
        qb, kb = divmod(g, KB)
        qs = slice(qb * Q_BLK, (qb + 1) * Q_BLK)
        ks = slice(kb * 128, (kb + 1) * 128)
        ps = psum.tile([128, 1024], F32, tag="scores", bufs=2, name="ps")
        nc.tensor.matmul(ps[:, 0:512], kt[0:64, ks], qt[0:64, qs])
        nc.tensor.matmul(ps[:, 512:1024], kt[64:128, ks], qt[64:128, qs])
        attn = sp.tile([128, 1024], BF16, tag="attn", bufs=6, name="attn")
        if not use_mask and qb > 0 and (kb in DVE_KBS):
            nc.vector.tensor_scalar(
                attn.bitcast(I16), ps, SCH_A, SCH_B,
                mybir.AluOpType.mult, mybir.AluOpType.add)
        else:
            nc.scalar.activation(
                attn, ps, EXP, scale=SCALE,
                bias=mb_sb[:, kb:kb + 1] if use_mask else 0.0)
        return attn

    def ctx_pair(pc, kb, attn, first, last):
        nc.tensor.matmul(pc[0:64, :], v_all[:, kb, 0:64],
                         attn[:, 0:512], start=first, stop=last,
                         skip_group_check=True)
        nc.tensor.matmul(pc[64:128, :], v_all[:, kb, 64:128],
                         attn[:, 512:1024], start=first, stop=last,
                         skip_group_check=True)

    fifo = [scores_block(g) for g in range(4)]
    pc = pd = None
    for u in range(NBLK // 2):
        g0, g1 = 2 * u, 2 * u + 1
        qb, kb0 = divmod(g0, KB)
        kb1 = kb0 + 1
        qs = slice(qb * Q_BLK, (qb + 1) * Q_BLK)
        if kb0 == 0:
            pc = psum.tile([128, 512], F32, tag="ctx", bufs=2, name="pc")
            pd = psum.tile([97, 512], F32, tag="den", bufs=2, name="pd")
        for stage in pending.pop(u, ()):
            stage()

        a0 = fifo.pop(0)
        ctx_pair(pc, kb0, a0, kb0 == 0, False)
        if g0 + 4 < NBLK:
            fifo.append(scores_block(g0 + 4))
        a1 = fifo.pop(0)
        ctx_pair(pc, kb1, a1, False, kb1 == KB - 1)
        if g1 + 4 < NBLK:
            fifo.append(scores_block(g1 + 4))

        # fused denominator slot: 4 concurrent M=1 matmuls, one per col
        # group; even block sums land on rows 0/32, odd block on 64/96
        for (row, att, sl) in ((0, a0, slice(0, 512)), (32, a0, slice(512, 1024)),
                               (64, a1, slice(0, 512)), (96, a1, slice(512, 1024))):
            nc.tensor.matmul(pd[row:row + 1, :], ones_col[:, 0:1], att[:, sl],
                             start=(kb0 == 0), stop=(kb1 == KB - 1),
                             skip_group_check=True,
                             tile_position=(0, row))

        if kb1 != KB - 1:
            continue

        # ---- qb tail ----
        # pd -> sbuf f16 (clamped so psum garbage in unused rows can't
        # poison the selector matmul with inf), then dsum = sel.T @ pd_sb
        # sums the even/odd halves per head; reciprocal + f16 cast feed
        # the PE broadcast. All emitted eagerly so the den-ring slots are
        # freed in allocation order; muls/oproj staggered into qb+1.
        pd_sb = sp.tile([97, 512], F16, tag="pd_sb", bufs=2)
        nc.vector.tensor_scalar(pd_sb, pd, 60000.0, -60000.0,
                                mybir.AluOpType.min, mybir.AluOpType.max)
        dsum = psum.tile([33, 512], F32, tag="den", bufs=2, name="dsum")
        nc.tensor.matmul(dsum[0:1, :], sel[:, 0:1], pd_sb,
                         skip_group_check=True, tile_position=(0, 0))
        nc.tensor.matmul(dsum[32:33, :], sel[:, 1:2], pd_sb,
                         skip_group_check=True, tile_position=(0, 32))
        rcp_f = sp.tile([33, 512], F32, tag="rcp_f", bufs=2)
        nc.vector.reciprocal_approx_fast(rcp_f, dsum)
        rcp = sp.tile([33, 512], F16, tag="rcp", bufs=2)
        nc.vector.tensor_copy(rcp, rcp_f)

        reps = [None, None]

        def bcast(h, rcp=rcp, reps=reps):
            r = 32 * h
            pr = psum.tile([128, 512], F32, tag="den", bufs=2, name=f"pr{h}")
            nc.tensor.matmul(pr, ones_rep[r:r + 1, :], rcp[r:r + 1, :])
            rep = sp.tile([128, 512], F32, tag="rep", bufs=2, name=f"rep{h}")
            nc.vector.tensor_copy(rep, pr)
            reps[h] = rep

        bcast(0)
        bcast(1)

        def muls(qs=qs, pc=pc, reps=reps):
            nc.vector.tensor_mul(ctxn[0:64, qs], pc[0:64, :], reps[0][0:64, :])
            nc.vector.tensor_mul(ctxn[64:128, qs], pc[64:128, :],
                                 reps[1][64:128, :])

        def oproj(i, qb=qb):
            sb = qb * (Q_BLK // 128) + i
            po = psum.tile([128, 512], F32, tag="ctx", bufs=2, name="po")
            nc.tensor.matmul(po, ctxn[:, sb * 128:(sb + 1) * 128], wo_sb)
            ob = sp.tile([128, 512], F32, tag="ob", bufs=3, name="ob")
            nc.vector.tensor_copy(ob, po)
            nc.sync.dma_start(d["out"].ap()[sb * 128:(sb + 1) * 128, :], ob)

        tail = [muls, lambda: oproj(0), lambda: oproj(1),
                lambda: oproj(2), lambda: oproj(3)]
        if qb == QB - 1:
            for t in tail:
                t()
        else:
            base = 16 * (qb + 1)
            for off, t in zip((4, 6, 8, 10, 12), tail):
                pending.setdefault(base + off, []).append(t)

    for u in sorted(pending):
        for stage in pending[u]:
            stage()


def build_program(S=4096, use_mask=False, use_bq=False, use_bk=False,
                  use_bv=False, enable_asserts=False):
    nc = bacc.Bacc("TRN2", target_bir_lowering=False, debug=False,
                   enable_asserts=enable_asserts, num_devices=N_CORES,
                   name="mha")
    d = {
        "xt": nc.dram_tensor("xt", [D_MODEL, S], BF16, kind="ExternalInput"),
        "wq": nc.dram_tensor("wq", [D_MODEL, DL], BF16, kind="ExternalInput"),
        "wk": nc.dram_tensor("wk", [D_MODEL, DL], BF16, kind="ExternalInput"),
        "wv": nc.dram_tensor("wv", [D_MODEL, DL], BF16, kind="ExternalInput"),
        "wo": nc.dram_tensor("wo", [DL, D_MODEL], BF16, kind="ExternalInput"),
        "out": nc.dram_tensor("out", [S, D_MODEL], F32, kind="ExternalOutput"),
    }
    if use_bq:
        d["bq"] = nc.dram_tensor("bq", [DL], F32, kind="ExternalInput")
    if use_bk:
        d["bk"] = nc.dram_tensor("bk", [DL], F32, kind="ExternalInput")
    if use_bv:
        d["bv"] = nc.dram_tensor("bv", [DL], F32, kind="ExternalInput")
    if use_mask:
        d["mb"] = nc.dram_tensor("mb", [128, S // 128], F32,
                                 kind="ExternalInput")
    with tile.TileContext(nc) as tc:
        with ExitStack() as ctx:
            build_kernel(ctx, tc, S, use_mask, use_bq, use_bk, use_bv, d)
    nc.compile()
    return nc


_cache = {}


def _program(key):
    if key not in _cache:
        _cache[key] = build_program(
            S=4096, use_mask=key[0], use_bq=key[1], use_bk=key[2],
            use_bv=key[3])
    return _cache[key]


def kernel(x, mask, Wq, bq, Wk, bk, Wv, bv, Wo, bo, _results_hook=None):
    x = np.asarray(x, np.float32)
    mask = np.asarray(mask)
    B, S, _ = x.shape
    use_mask = bool((mask == 0).any())
    use_bq = bool(np.asarray(bq).any())
    use_bk = bool(np.asarray(bk).any())
    use_bv = bool(np.asarray(bv).any())
    nc = _program((use_mask, use_bq, use_bk, use_bv))

    in_maps = []
    for c in range(N_CORES):
        b, j = divmod(c, N_CORES // B)
        ds = slice(j * DL, (j + 1) * DL)
        m = {
            "xt": np.ascontiguousarray(x[b].T).astype(ml_dtypes.bfloat16),
            "wq": np.ascontiguousarray(Wq[:, ds]).astype(ml_dtypes.bfloat16),
            "wk": np.ascontiguousarray(Wk[:, ds]).astype(ml_dtypes.bfloat16),
            "wv": np.ascontiguousarray(Wv[:, ds]).astype(ml_dtypes.bfloat16),
            "wo": np.ascontiguousarray(Wo[ds, :]).astype(ml_dtypes.bfloat16),
        }
        if use_bq:
            m["bq"] = np.ascontiguousarray(bq[ds], dtype=np.float32)
        if use_bk:
            m["bk"] = np.ascontiguousarray(bk[ds], dtype=np.float32)
        if use_bv:
            m["bv"] = np.ascontiguousarray(bv[ds], dtype=np.float32)
        if use_mask:
            mb = np.where(np.asarray(mask[b]) == 0, -1e9, 0.0).astype(np.float32)
            m["mb"] = np.ascontiguousarray(mb.reshape(S // 128, 128).T)
        in_maps.append(m)

    res = run_bass_kernel_spmd(nc, in_maps, core_ids=list(range(N_CORES)))
    if _results_hook is not None:
        _results_hook(res)
    out = np.zeros((B, S, D_MODEL), np.float32)
    for c in range(N_CORES):
        b = c // (N_CORES // B)
        out[b] += res.results[c]["out"]
    out += np.asarray(bo, np.float32)
    return out


# revision 9
# speedup vs baseline: 1.1413x; 1.0137x over previous
"""Multi-head self-attention (B=2, S=4096, D=512, H=8, Dk=64) on 8 TRN2 cores.

Sharding: data-parallel over batch x head-parallel. Core c handles batch
c//4 and head pair (2*(c%4), 2*(c%4)+1). Each core computes Q/K/V
projections for its 128 model dims, full attention for its two heads, and
a partial output projection against its 128 rows of Wo. The host sums the
four partial outputs per batch and adds bo.

x arrives host-transposed as xT [512, S] bf16, streamed in 512-token
chunks; chunk-0 projections run up front and the remaining chunks'
K/Q/V projections are staggered into query-block 0's key loop so
attention (and the exp stream, the critical resource) starts as soon as
chunk 0 lands instead of after all projections.

The softmax exp is split across TWO engines: the Scalar (ACT) engine
computes exact exp for most key blocks, and the Vector (DVE) engine
computes a Schraudolph-style exp2 approximation (one tensor_scalar op:
round(s*A + B) -> int16, bit-cast as bf16) for DVE_QB blocks per query
block. ACT throughput is (N+352)/1.2 ns per [128, N] block and exp
exists only on ACT, so offloading ~40% of blocks to the otherwise-idle
DVE removes the single-engine exp floor (~294 us). The approximation
carries +-3% per-weight error; softmax renormalization cancels the mean
and the verified end-to-end rel err is ~1.3e-2 (gate 2e-2).

On-core layout (bf16 operands, fp32 psum accumulation):
  xT   [d, s]   bf16, host-transposed        (rhs for Q/K, lhsT for V)
  QT/KT [128, S] bf16, head0 in partitions 0:64, head1 in 64:128
  V    [s, 128] bf16, head0 in cols 0:64, head1 in 64:128 (lhsT for ctx)
  scoresT[k, q] fp32 psum from row-paired bf16 matmuls (K=64/head)
  attnT = exp(scoresT/8) bf16, per [128, 1024] block on ACT or DVE
  ctxT [d, q] fp32 psum, col-paired over k blocks; denominators from
  ones-vector matmuls into psum rows 0/32; reciprocal_approx_fast reads
  them straight from psum; normalization via fp32 PE broadcast.
"""

import numpy as np
import ml_dtypes
from contextlib import ExitStack

import concourse.bass as bass
import concourse.tile as tile
from concourse import bacc, mybir
from concourse.bass_utils import run_bass_kernel_spmd
from concourse.tile_rust import add_dep_helper

F32 = mybir.dt.float32
F16 = mybir.dt.float16
BF16 = mybir.dt.bfloat16
I16 = mybir.dt.int16
EXP = mybir.ActivationFunctionType.Exp

D_MODEL = 512
N_HEADS = 8
D_K = 64
N_CORES = 8
DL = 128          # local model dims per core (2 heads)
Q_BLK = 512       # query block (free dim of scores matmuls)
SCALE = 1.0 / np.sqrt(D_K).item()

# Schraudolph exp2 on DVE: exp(s/8) ~ bf16-bitcast(int16(round(s*A + B)))
LOG2E = 1.4426950408889634
SCH_A = 128.0 * LOG2E * SCALE
SCH_B = 128.0 * (127.0 - 0.0434)

# key blocks per query block handled by the DVE exp path (odd kbs only:
# even blocks stay on ACT so the even psum-ring slot always frees in time;
# spread over the whole range so ACT never runs >3 consecutive blocks)
DVE_KBS = tuple(k for k in range(1, 32, 2) if k not in (1, 11, 21))  # 13


def build_kernel(ctx, tc, S, use_mask, use_bq, use_bk, use_bv, d):
    nc = tc.nc
    SB = S // 128    # s blocks of 128
    QB = S // Q_BLK  # query blocks of 512
    KB = S // 128    # key blocks of 128
    CHUNK = 512
    NCH = S // CHUNK

    sp = ctx.enter_context(tc.tile_pool(name="sp", bufs=1))
    psum = ctx.enter_context(tc.tile_pool(name="psum", bufs=1, space="PSUM"))
    # psum budget (8 banks): scores 2x[128,1024]=4, ctx 2x[128,512]=2,
    # den 2x[<=1 bank]=2. All other matmul outputs share the ctx/den tags.

    # ---- constants ----
    ones_f = sp.tile([128, 1], F32, tag="ones_f")
    nc.vector.memset(ones_f, 1.0)
    ones_col = sp.tile([128, 1], BF16, tag="ones_col")  # lhsT of denominator mms
    nc.vector.tensor_copy(ones_col, ones_f)
    ones_rep = sp.tile([33, 128], F16, tag="ones_rep")  # lhsT of broadcast mms
    nc.vector.memset(ones_rep, 1.0)
    # selector for the denominator-halves sum: dsum = sel.T @ pd_sb with
    # col 0 picking rows {0,64} (head 0) and col 1 rows {32,96} (head 1)
    sel = sp.tile([97, 2], F16, tag="sel")
    nc.vector.memset(sel, 0.0)
    nc.vector.memset(sel[0:1, 0:1], 1.0)
    nc.vector.memset(sel[64:65, 0:1], 1.0)
    nc.vector.memset(sel[32:33, 1:2], 1.0)
    nc.vector.memset(sel[96:97, 1:2], 1.0)

    # ---- DMA in: weights first (small; needed by every projection), then
    # x (host-transposed bf16) in 512-token chunks so chunk-0 projections
    # and the first exp start as early as possible. ----
    wq_sb = sp.tile([128, 4, 128], BF16, tag="wq")
    nc.sync.dma_start(wq_sb, d["wq"].ap().rearrange("(t p) d -> p t d", p=128))
    wk_sb = sp.tile([128, 4, 128], BF16, tag="wk")
    nc.sync.dma_start(wk_sb, d["wk"].ap().rearrange("(t p) d -> p t d", p=128))
    wv_sb = sp.tile([128, 4, 128], BF16, tag="wv")
    nc.sync.dma_start(wv_sb, d["wv"].ap().rearrange("(t p) d -> p t d", p=128))
    wo_sb = sp.tile([128, 512], BF16, tag="wo")
    nc.sync.dma_start(wo_sb, d["wo"].ap())
    if use_bq:
        bq_sb = sp.tile([128, 1], F32, tag="bq")
        nc.sync.dma_start(bq_sb, d["bq"].ap()[:, None])
    if use_bk:
        bk_sb = sp.tile([128, 1], F32, tag="bk")
        nc.sync.dma_start(bk_sb, d["bk"].ap()[:, None])
    if use_bv:
        bv_sb = sp.tile([1, 128], F32, tag="bv")
        nc.sync.dma_start(bv_sb, d["bv"].ap()[None, :])
        ones_row = sp.tile([1, 128], F32, tag="ones_row")
        nc.vector.memset(ones_row, 1.0)
    if use_mask:
        mb_sb = sp.tile([128, KB], F32, tag="mb")
        nc.sync.dma_start(mb_sb, d["mb"].ap())

    xt = sp.tile([128, 4, S], BF16, tag="xt")
    xsrc = d["xt"].ap().rearrange("(t p) s -> p t s", p=128)
    for c in range(NCH):
        cs = slice(c * CHUNK, (c + 1) * CHUNK)
        nc.sync.dma_start(xt[:, :, cs], xsrc[:, :, cs])

    # ---- PE warm-up: the HAM clock gate needs ~3.4us of sustained matmul
    # activity to lift the PE from 1.2 to 2.4 GHz; run throwaway matmuls
    # while x streams in so the projections start at full clock. ----
    scratch = sp.tile([128, 512], BF16, tag="scratch")
    nc.vector.memset(scratch, 0.0)
    for _ in range(20):
        pw = psum.tile([33, 512], F32, tag="den", bufs=2, name="pw")
        nc.tensor.matmul(pw[0:1, :], scratch[:, 0:1], scratch)

    # ---- projections ----
    qt = sp.tile([128, S], BF16, tag="qt")
    kt = sp.tile([128, S], BF16, tag="kt")
    v_all = sp.tile([128, SB, 128], BF16, tag="v")

    def proj_qk(dst, w_sb, b_sb, c):
        # one 512-token sub-chunk of the Q or K projection
        pp = psum.tile([128, 512], F32, tag="ctx", bufs=2, name="pp")
        for t in range(4):
            nc.tensor.matmul(pp, w_sb[:, t, :], xt[:, t, c * 512:(c + 1) * 512],
                             start=(t == 0), stop=(t == 3))
        out = dst[:, c * 512:(c + 1) * 512]
        if b_sb is not None:
            nc.vector.tensor_scalar_add(out, pp, b_sb[:, 0:1])
        else:
            nc.vector.tensor_copy(out, pp)

    def proj_v(sb):
        pv = psum.tile([128, 128], F32, tag="den", bufs=2, name="pv")
        for t in range(4):
            nc.tensor.matmul(pv, xt[:, t, sb * 128:(sb + 1) * 128], wv_sb[:, t, :],
                             start=(t == 0), stop=(t == 3 and not use_bv))
        if use_bv:
            nc.tensor.matmul(pv, ones_row[0:1, :], bv_sb[0:1, :],
                             start=False, stop=True)
        nc.vector.tensor_copy(v_all[:, sb, :], pv)

    bkk = bk_sb if use_bk else None
    bqq = bq_sb if use_bq else None

    # chunk 0 up front: everything query-block 0 needs to start
    proj_qk(kt, wk_sb, bkk, 0)
    proj_qk(qt, wq_sb, bqq, 0)
    for sb in range(4):
        proj_v(sb)

    # chunks 1..NCH-1 staggered into qb0's units. Chunk c's K/V blocks are
    # first consumed at global block 4c (scores emitted at unit 2c-2), so
    # emit them in the two units before that, DMA-gated by Tile deps.
    pending = {}

    def make_chunk_stage(c, part):
        if part == 0:
            return lambda: (proj_qk(kt, wk_sb, bkk, c),
                            proj_v(4 * c), proj_v(4 * c + 1))
        return lambda: (proj_v(4 * c + 2), proj_v(4 * c + 3),
                        proj_qk(qt, wq_sb, bqq, c))

    for c in range(1, NCH):
        u = 2 * (c - 1)
        pending.setdefault(u, []).append(make_chunk_stage(c, 0))
        pending.setdefault(u + 1, []).append(make_chunk_stage(c, 1))

    # ---- attention: one flat loop over 2-block units ----
    # Per unit (blocks g0=2u, g1=2u+1): ctx pair for g0, scores pair for
    # g0+4, ctx pair for g1, scores pair for g1+4, then ONE fused 4-way
    # denominator slot (M=1 matmuls on col groups 0/32/64/96 of pd).
    # The 4-block scores lookahead keeps exp latency off the ctx critical
    # path; even blocks always use ACT so the even psum-ring slot is
    # ready when scores(g+4) needs it.
    NBLK = QB * KB
    ctxn = sp.tile([128, S], BF16, tag="ctxn")

    def scores_block(g):
        qb, kb = divmod(g, KB)
        qs = slice(qb * Q_BLK, (qb + 1) * Q_BLK)
        ks = slice(kb * 128, (kb + 1) * 128)
        ps = psum.tile([128, 1024], F32, tag="scores", bufs=2, name="ps")
        nc.tensor.matmul(ps[:, 0:512], kt[0:64, ks], qt[0:64, qs])
        nc.tensor.matmul(ps[:, 512:1024], kt[64:128, ks], qt[64:128, qs])
        attn = sp.tile([128, 1024], BF16, tag="attn", bufs=6, name="attn")
        if not use_mask and qb > 0 and (kb in DVE_KBS):
            nc.vector.tensor_scalar(
                attn.bitcast(I16), ps, SCH_A, SCH_B,
                mybir.AluOpType.mult, mybir.AluOpType.add)
        else:
            nc.scalar.activation(
                attn, ps, EXP, scale=SCALE,
                bias=mb_sb[:, kb:kb + 1] if use_mask else 0.0)
        return attn

    def ctx_pair(pc, kb, attn, first, last):
        nc.tensor.matmul(pc[0:64, :], v_all[:, kb, 0:64],
                         attn[:, 0:512], start=first, stop=last,
                         skip_group_check=True)
        nc.tensor.matmul(pc[64:128, :], v_all[:, kb, 64:128],
                         attn[:, 512:1024], start=first, stop=last,
                         skip_group_check=True)

    fifo = [scores_block(g) for g in range(4)]
    pc = pd = None
    for u in range(NBLK // 2):
        g0, g1 = 2 * u, 2 * u + 1
        qb, kb0 = divmod(g0, KB)
        kb1 = kb0 + 1
        qs = slice(qb * Q_BLK, (qb + 1) * Q_BLK)
        if kb0 == 0:
            pc = psum.tile([128, 512], F32, tag="ctx", bufs=2, name="pc")
            pd = psum.tile([97, 512], F32, tag="den", bufs=2, name="pd")
        for stage in pending.pop(u, ()):
            stage()

        a0 = fifo.pop(0)
        ctx_pair(pc, kb0, a0, kb0 == 0, False)
        if g0 + 4 < NBLK:
            fifo.append(scores_block(g0 + 4))
        a1 = fifo.pop(0)
        ctx_pair(pc, kb1, a1, False, kb1 == KB - 1)
        if g1 + 4 < NBLK:
            fifo.append(scores_block(g1 + 4))

        # fused denominator slot: 4 concurrent M=1 matmuls, one per col
        # group; even block sums land on rows 0/32, odd block on 64/96
        for (row, att, sl) in ((0, a0, slice(0, 512)), (32, a0, slice(512, 1024)),
                               (64, a1, slice(0, 512)), (96, a1, slice(512, 1024))):
            nc.tensor.matmul(pd[row:row + 1, :], ones_col[:, 0:1], att[:, sl],
                             start=(kb0 == 0), stop=(kb1 == KB - 1),
                             skip_group_check=True,
                             tile_position=(0, row))

        if kb1 != KB - 1:
            continue

        # ---- qb tail ----
        # pd -> sbuf f16 (clamped so psum garbage in unused rows can't
        # poison the selector matmul with inf), then dsum = sel.T @ pd_sb
        # sums the even/odd halves per head; reciprocal + f16 cast feed
        # the PE broadcast. All emitted eagerly so the den-ring slots are
        # freed in allocation order; muls/oproj staggered into qb+1.
        pd_sb = sp.tile([97, 512], F16, tag="pd_sb", bufs=2)
        nc.vector.tensor_scalar(pd_sb, pd, 60000.0, -60000.0,
                                mybir.AluOpType.min, mybir.AluOpType.max)
        dsum = psum.tile([33, 512], F32, tag="den", bufs=2, name="dsum")
        nc.tensor.matmul(dsum[0:1, :], sel[:, 0:1], pd_sb,
                         skip_group_check=True, tile_position=(0, 0))
        nc.tensor.matmul(dsum[32:33, :], sel[:, 1:2], pd_sb,
                         skip_group_check=True, tile_position=(0, 32))
        rcp_f = sp.tile([33, 512], F32, tag="rcp_f", bufs=2)
        nc.vector.reciprocal_approx_fast(rcp_f, dsum)
        rcp = sp.tile([33, 512], F16, tag="rcp", bufs=2)
        nc.vector.tensor_copy(rcp, rcp_f)

        reps = [None, None]

        def bcast(h, rcp=rcp, reps=reps):
            r = 32 * h
            pr = psum.tile([128, 512], F32, tag="den", bufs=2, name=f"pr{h}")
            nc.tensor.matmul(pr, ones_rep[r:r + 1, :], rcp[r:r + 1, :])
            rep = sp.tile([128, 512], F32, tag="rep", bufs=2, name=f"rep{h}")
            nc.vector.tensor_copy(rep, pr)
            reps[h] = rep

        bcast(0)
        bcast(1)

        def muls(qs=qs, pc=pc, reps=reps):
            nc.vector.tensor_mul(ctxn[0:64, qs], pc[0:64, :], reps[0][0:64, :])
            nc.vector.tensor_mul(ctxn[64:128, qs], pc[64:128, :],
                                 reps[1][64:128, :])

        def oproj(i, qb=qb):
            sb = qb * (Q_BLK // 128) + i
            po = psum.tile([128, 512], F32, tag="ctx", bufs=2, name="po")
            nc.tensor.matmul(po, ctxn[:, sb * 128:(sb + 1) * 128], wo_sb)
            ob = sp.tile([128, 512], F32, tag="ob", bufs=3, name="ob")
            nc.vector.tensor_copy(ob, po)
            nc.sync.dma_start(d["out"].ap()[sb * 128:(sb + 1) * 128, :], ob)

        tail = [muls, lambda: oproj(0), lambda: oproj(1),
                lambda: oproj(2), lambda: oproj(3)]
        if qb == QB - 1:
            for t in tail:
                t()
        else:
            base = 16 * (qb + 1)
            for off, t in zip((4, 6, 8, 10, 12), tail):
                pending.setdefault(base + off, []).append(t)

    for u in sorted(pending):
        for stage in pending[u]:
            stage()


def build_program(S=4096, use_mask=False, use_bq=False, use_bk=False,
                  use_bv=False, enable_asserts=False):
    nc = bacc.Bacc("TRN2", target_bir_lowering=False, debug=False,
                   enable_asserts=enable_asserts, num_devices=N_CORES,
                   name="mha")
    d = {
        "xt": nc.dram_tensor("xt", [D_MODEL, S], BF16, kind="ExternalInput"),
        "wq": nc.dram_tensor("wq", [D_MODEL, DL], BF16, kind="ExternalInput"),
        "wk": nc.dram_tensor("wk", [D_MODEL, DL], BF16, kind="ExternalInput"),
        "wv": nc.dram_tensor("wv", [D_MODEL, DL], BF16, kind="ExternalInput"),
        "wo": nc.dram_tensor("wo", [DL, D_MODEL], BF16, kind="ExternalInput"),
        "out": nc.dram_tensor("out", [S, D_MODEL], F32, kind="ExternalOutput"),
    }
    if use_bq:
        d["bq"] = nc.dram_tensor("bq", [DL], F32, kind="ExternalInput")
    if use_bk:
        d["bk"] = nc.dram_tensor("bk", [DL], F32, kind="ExternalInput")
    if use_bv:
        d["bv"] = nc.dram_tensor("bv", [DL], F32, kind="ExternalInput")
    if use_mask:
        d["mb"] = nc.dram_tensor("mb", [128, S // 128], F32,
                                 kind="ExternalInput")
    with tile.TileContext(nc) as tc:
        with ExitStack() as ctx:
            build_kernel(ctx, tc, S, use_mask, use_bq, use_bk, use_bv, d)
    nc.compile()
    return nc


_cache = {}


def _program(key):
    if key not in _cache:
        _cache[key] = build_program(
            S=4096, use_mask=key[0], use_bq=key[1], use_bk=key[2],
            use_bv=key[3])
    return _cache[key]


def kernel(x, mask, Wq, bq, Wk, bk, Wv, bv, Wo, bo, _results_hook=None):
    x = np.asarray(x, np.float32)
    mask = np.asarray(mask)
    B, S, _ = x.shape
    use_mask = bool((mask == 0).any())
    use_bq = bool(np.asarray(bq).any())
    use_bk = bool(np.asarray(bk).any())
    use_bv = bool(np.asarray(bv).any())
    nc = _program((use_mask, use_bq, use_bk, use_bv))

    in_maps = []
    for c in range(N_CORES):
        b, j = divmod(c, N_CORES // B)
        ds = slice(j * DL, (j + 1) * DL)
        m = {
            "xt": np.ascontiguousarray(x[b].T).astype(ml_dtypes.bfloat16),
            "wq": np.ascontiguousarray(Wq[:, ds]).astype(ml_dtypes.bfloat16),
            "wk": np.ascontiguousarray(Wk[:, ds]).astype(ml_dtypes.bfloat16),
            "wv": np.ascontiguousarray(Wv[:, ds]).astype(ml_dtypes.bfloat16),
            "wo": np.ascontiguousarray(Wo[ds, :]).astype(ml_dtypes.bfloat16),
        }
        if use_bq:
            m["bq"] = np.ascontiguousarray(bq[ds], dtype=np.float32)
        if use_bk:
            m["bk"] = np.ascontiguousarray(bk[ds], dtype=np.float32)
        if use_bv:
            m["bv"] = np.ascontiguousarray(bv[ds], dtype=np.float32)
        if use_mask:
            mb = np.where(np.asarray(mask[b]) == 0, -1e9, 0.0).astype(np.float32)
            m["mb"] = np.ascontiguousarray(mb.reshape(S // 128, 128).T)
        in_maps.append(m)

    res = run_bass_kernel_spmd(nc, in_maps, core_ids=list(range(N_CORES)))
    if _results_hook is not None:
        _results_hook(res)
    out = np.zeros((B, S, D_MODEL), np.float32)
    for c in range(N_CORES):
        b = c // (N_CORES // B)
        out[b] += res.results[c]["out"]
    out += np.asarray(bo, np.float32)
    return out
